# revision 14
# baseline (speedup 1.0000x reference)
"""Trainium2 Bass kernel for nn_Net4 (hypernetwork RNN scan).

Model (per step t, per batch row b):
  h1 = sigmoid(m @ A1 + pre1[t])          A1 = W_enc_w[:64]
  h2 = sigmoid(m @ B1 + pre2[t])          B1 = b_enc_w[:64]
  Wm = (h1 @ W_dec_w + W_dec_b).reshape(64,64)
  bm = h2 @ b_dec_w + b_dec_b
  m' = sigmoid(Wm @ m + bm)
  loss[t] = (logsumexp(m'@dec_w+dec_b) - (m'@dec_w+dec_b)[y]) / ln2

pre1/pre2 are the window-dependent parts, precomputed on device via a
shifted-embedding matmul, stored interleaved in preC (bf16).  The
bilinear Wm@m is reassociated as
  a[b,i] = sum_h h1[b,h] * T[b,h,i],  T[b,h,i] = sum_j W2r[h,i,j] m[b,j]

Scan critical cycle (all bf16):
  sigma(a) -> m_seq -> {g-MMs, 32 T-MMs} -> T copies -> a-MMs -> sigma(a)
The pre1/pre2 bias is injected into g_ps by an identity matmul (so one
sigmoid covers h1-dup + h2), and a_ps is duplicated into both partition
halves by col-tiled twin a-MMs (so one sigmoid produces the duplicated
m layout the next step's matmuls need).

Sharding: batch rows 2k,2k+1 -> core k; zero cross-core communication.
"""

import os
import sys
import numpy as np

sys.path.insert(0, "/opt/trn_rl_repo")

import concourse.bass as bass
import concourse.bacc as bacc
import concourse.mybir as mybir
import concourse.tile as tile
from concourse.bass_utils import run_bass_kernel_spmd

import ml_dtypes

BF16 = ml_dtypes.bfloat16
F8 = ml_dtypes.float8_e4m3

Cin, E, L, M, H, Cout = 256, 16, 64, 64, 64, 256
B, N = 16, 2048
D = M + L * E  # 1088
NCORES = 8
BL = B // NCORES  # 2 batch rows per core
NB = N * BL       # 4096 (t,b) pairs per core
TAU = N + L - 8   # e8 time length: tau in [0, 2104)
E8COLS = TAU * BL  # 4208

F32 = mybir.dt.float32
BF16_DT = mybir.dt.bfloat16
F8_DT = mybir.dt.float8e4
AF = mybir.ActivationFunctionType

_cache = {}


def _build_nc(unroll=16, staggered=False, has_wdb=False, has_bdb=False,
              has_decb=False, sim_closers=None):
    if sim_closers is None:
        sim_closers = bool(os.environ.get("KERNEL_SIM_CLOSERS"))
    nc = bacc.Bacc("TRN2", target_bir_lowering=False, debug=True)

    # ---- DRAM parameters (per-core inputs) ----
    def P(name, shape, dt):
        return nc.declare_dram_parameter(name, list(shape), dt, isOutput=False)

    e8_d = P("e8", (128, E8COLS), BF16_DT)
    wpre1_d = P("wpre1", (128, 8 * 128), BF16_DT)
    wpre2_d = P("wpre2", (128, 8 * 64), BF16_DT)
    bias1_d = P("bias1", (1, 128), BF16_DT)   # [W_enc_b | W_enc_b]
    bias2_d = P("bias2", (1, 64), BF16_DT)    # b_enc_b
    a1b1_d = P("a1b1", (128, 192), F8_DT)
    wstatT_d = P("wstatT", (128, 16 * 128), F8_DT)
    ident_d = P("ident", (128, 128), BF16_DT)
    bwdup_d = P("bwdup", (64, 128), BF16_DT)  # [b_dec_w | b_dec_w]
    dec_d = P("decbf", (64, 256), BF16_DT)    # dec_w
    gaug_d = P("gaug", (65, NB), BF16_DT)     # dec_wT[y] rows + dec_b[y] row
    ones65_d = P("ones65", (65, 1), BF16_DT)
    ones128_d = P("ones128", (128, 1), BF16_DT)
    if has_wdb:
        wbdup_d = P("wbdup", (64, 128), BF16_DT)  # W_dec_b resh [j,i] dup
    if has_bdb:
        bdb_d = P("bdb", (64, 1), F32)        # b_dec_b (sigmoid bias)
    if has_decb:
        decb_d = P("decb", (128, 2), F32)     # dec_b halves (exp bias)
    out_d = nc.declare_dram_parameter("out", [1, NB], F32, isOutput=True)
    dbg = bool(os.environ.get("KERNEL_DEBUG"))
    if dbg:
        outm_d = nc.declare_dram_parameter("outm", [64, NB + BL], BF16_DT, isOutput=True)
        outp_d = nc.declare_dram_parameter("outp", [128, N, 4], BF16_DT, isOutput=True)

    with tile.TileContext(nc) as tc:
        with (
            tc.tile_pool(name="persist", bufs=1) as pp,
            tc.tile_pool(name="psum", bufs=2, space="PSUM") as psp,
        ):
            e8 = pp.tile([128, E8COLS], BF16_DT)
            wpre1 = pp.tile([128, 8 * 128], BF16_DT)
            wpre2 = pp.tile([128, 8 * 64], BF16_DT)
            bias1 = pp.tile([1, 128], BF16_DT)
            bias2 = pp.tile([1, 64], BF16_DT)
            a1b1 = pp.tile([128, 192], F8_DT)
            wstatT = pp.tile([128, 16 * 128], F8_DT)
            ident = pp.tile([128, 128], BF16_DT)
            bwdup = pp.tile([64, 128], BF16_DT)
            decbf = pp.tile([64, 256], BF16_DT)
            gaug = pp.tile([65, NB], BF16_DT)
            ones65 = pp.tile([65, 1], BF16_DT)
            ones128 = pp.tile([128, 1], BF16_DT)

            loads = [
                (e8, e8_d), (wpre1, wpre1_d), (wpre2, wpre2_d),
                (bias1, bias1_d), (bias2, bias2_d), (a1b1, a1b1_d),
                (wstatT, wstatT_d), (ident, ident_d), (bwdup, bwdup_d),
                (decbf, dec_d), (gaug, gaug_d),
                (ones65, ones65_d), (ones128, ones128_d),
            ]
            if has_wdb:
                wbdup = pp.tile([64, 128], BF16_DT)
                loads.append((wbdup, wbdup_d))
            if has_bdb:
                bdb = pp.tile([64, 1], F32)
                loads.append((bdb, bdb_d))
            if has_decb:
                decb = pp.tile([128, 2], F32)
                loads.append((decb, decb_d))
            for sb, dr in loads:
                nc.default_dma_engine.dma_start(sb[:], dr[:])

            # preC: per step 4 cols: [pre1_dup(2) | pre2(2)] (bf16)
            preC = pp.tile([128, N, 4], BF16_DT)
            m_seq = pp.tile([64, NB + BL], BF16_DT)
            m_blk = pp.tile([128, 4], F8_DT)
            onerow = pp.tile([1, 512], BF16_DT)
            zrow = pp.tile([1, 128], BF16_DT)
            scrap = pp.tile([1, 2], F32)

            nc.vector.memset(m_seq[:, 0:BL], 0.0)
            nc.vector.memset(m_blk[:], 0.0)
            nc.vector.memset(preC[64:128, :, 2:4], 0.0)
            nc.vector.memset(onerow[:], 1.0)
            nc.vector.memset(zrow[:], 0.0)
            nc.vector.memset(scrap[:], 0.0)

            # ---- precompute pre1/pre2 -> preC (interleaved, bf16) ----
            for n in range(8):
                ps1 = psp.tile([128, 256, 2], F32, tag="pps")
                nc.tensor.matmul(ps1[:], bias1[:], onerow[:],
                                 start=True, stop=False)
                for c in range(8):
                    nc.tensor.matmul(
                        ps1[:],
                        wpre1[:, c * 128:(c + 1) * 128],
                        e8[:, 16 * c + 512 * n: 16 * c + 512 * n + 512],
                        start=False, stop=(c == 7),
                    )
                nc.vector.tensor_copy(
                    preC[:, 256 * n: 256 * (n + 1), 0:2], ps1[:])
                ps2 = psp.tile([64, 256, 2], F32, tag="pps")
                nc.tensor.matmul(ps2[:], bias2[:], onerow[:],
                                 start=True, stop=False)
                for c in range(8):
                    nc.tensor.matmul(
                        ps2[:],
                        wpre2[:, c * 64:(c + 1) * 64],
                        e8[:, 16 * c + 512 * n: 16 * c + 512 * n + 512],
                        start=False, stop=(c == 7),
                    )
                nc.vector.tensor_copy(
                    preC[0:64, 256 * n: 256 * (n + 1), 2:4], ps2[:])

            # dummy sigmoid so the act-table load is hoisted out of the loop
            nc.scalar.activation(scrap[:], scrap[:], AF.Sigmoid)

            # ---- the scan ----
            with (
                tc.tile_pool(name="scan_sb", bufs=2) as wp,
                tc.For_i(0, N, unroll, staggered_reset=staggered,
                         hint_engines=(mybir.EngineType.PE,)) as iv,
            ):
                for k in range(unroll):
                    tcol = (iv + k) * BL
                    g_ps = psp.tile([128, 4], F32, tag="g_ps", bufs=1)
                    T_psA = psp.tile([128, 9, 4], F32, tag="T_psA", bufs=1)
                    T_psB = psp.tile([128, 6, 4], F32, tag="T_psB", bufs=1)
                    T_psC = psp.tile([128, 1, 4], F32, tag="T_psC", bufs=1)
                    a_ps = psp.tile([64, BL], F32, tag="a_ps")
                    hbuf = wp.tile([128, 4], BF16_DT, tag="hbuf")
                    tsb = wp.tile([128, 16, 4], BF16_DT, tag="tsb")

                    # g = pre (identity MM) + [A1|A1]m (rows 0-63 dup) + A2 m
                    nc.tensor.matmul(g_ps[:], ident[:],
                                     preC[:, bass.ds(iv + k, 1), :],
                                     start=True, stop=False)
                    nc.tensor.matmul(g_ps[0:64, 2:4], a1b1[0:64, 128:192],
                                     m_blk[0:64, 0:2],
                                     start=False, stop=False,
                                     skip_group_check=True,
                                     tile_position=(0, 0))
                    nc.tensor.matmul(g_ps[:, 0:2], a1b1[0:64, 0:128],
                                     m_blk[0:64, 0:2],
                                     start=False, stop=not sim_closers,
                                     skip_group_check=True,
                                     tile_position=(0, 0))
                    if sim_closers:
                        nc.tensor.matmul(g_ps[:], zrow[:, 0:128],
                                         onerow[:, 0:4],
                                         start=False, stop=True)
                    # T chunks: one full-array MM per 128x128 stationary;
                    # rhs is block-diag so out cols 0:2 = even chunk (rows
                    # 0-63 of wstatT), cols 2:4 = odd chunk (rows 64-127)
                    for p2 in range(16):
                        Tp, pi = ((T_psA, p2) if p2 < 9 else
                                  (T_psB, p2 - 9) if p2 < 15 else
                                  (T_psC, p2 - 15))
                        nc.tensor.matmul(
                            Tp[:, pi, :],
                            wstatT[:, p2 * 128:(p2 + 1) * 128],
                            m_blk[:],
                            start=True, stop=True)
                    # h = sigmoid(g) : [h1 dup | h2] in one shot
                    nc.scalar.activation(hbuf[:], g_ps[:], AF.Sigmoid)
                    # T -> SBUF (bf16), layout-preserving
                    nc.vector.tensor_copy(tsb[:, 0:9, :], T_psA[:])
                    nc.scalar.activation(tsb[:, 9:15, :], T_psB[:], AF.Copy)
                    nc.vector.tensor_copy(tsb[:, 15:16, :], T_psC[:])
                    # a = bw@h2 (+ wb@m) + sum_h h1*T, dup'd into both halves
                    nc.tensor.matmul(a_ps[:], bwdup[:, 0:64], hbuf[0:64, 2:4],
                                     start=True, stop=False)
                    if has_wdb:
                        nc.tensor.matmul(a_ps[:], wbdup[:, 0:64],
                                         m_blk[0:64, 0:2],
                                         start=False, stop=False)
                    for b in range(BL):
                        nc.tensor.matmul(a_ps[0:32, b: b + 1],
                                         tsb[0:64, :, b:4:2],
                                         hbuf[0:64, b: b + 1],
                                         start=False, stop=False,
                                         skip_group_check=True,
                                         tile_position=(0, 0))
                        nc.tensor.matmul(a_ps[32:64, b: b + 1],
                                         tsb[64:128, :, b:4:2],
                                         hbuf[64:128, b: b + 1],
                                         start=False,
                                         stop=(b == BL - 1 and not sim_closers),
                                         skip_group_check=True,
                                         tile_position=(64, 32))
                    if sim_closers:
                        nc.tensor.matmul(a_ps[:], zrow[:, 0:64], onerow[:, 0:2],
                                         start=False, stop=True)
                    # m' = sigmoid(a), duplicated halves in one call
                    if has_bdb:
                        nc.scalar.activation(m_blk[0:64, 0:2], a_ps[:],
                                             AF.Sigmoid, bias=bdb[0:64, :])
                    else:
                        nc.scalar.activation(m_blk[0:64, 0:2], a_ps[:],
                                             AF.Sigmoid)
                    nc.vector.tensor_copy(m_blk[64:128, 2:4], m_blk[0:64, 0:2])
                    nc.vector.tensor_copy(m_seq[:, bass.ds(tcol + BL, BL)],
                                          m_blk[0:64, 0:2])

            # ---- bulk loss ----
            lse = pp.tile([1, NB], F32)
            paug = pp.tile([65, NB], BF16_DT)
            loss = pp.tile([1, NB], F32)
            nc.vector.tensor_copy(paug[64:65, :], gaug[64:65, :])
            nc.vector.tensor_tensor(paug[0:64, :], gaug[0:64, :],
                                    m_seq[:, BL: NB + BL],
                                    mybir.AluOpType.mult)
            with tc.tile_pool(name="bulk", bufs=2) as bp:
                for tcn in range(8):
                    sl = slice(512 * tcn, 512 * (tcn + 1))
                    se_ps = psp.tile([1, 512], F32, tag="pps")
                    for half in range(2):
                        lg_ps = psp.tile([128, 512], F32, tag="pps")
                        exps = bp.tile([128, 512], BF16_DT, tag="exps")
                        nc.tensor.matmul(
                            lg_ps[:],
                            decbf[:, half * 128:(half + 1) * 128],
                            m_seq[:, BL + 512 * tcn: BL + 512 * (tcn + 1)],
                            start=True, stop=True)
                        if has_decb:
                            nc.scalar.activation(exps[:], lg_ps[:], AF.Exp,
                                                 bias=decb[:, half:half + 1])
                        else:
                            nc.scalar.activation(exps[:], lg_ps[:], AF.Exp)
                        nc.tensor.matmul(se_ps[:], ones128[:], exps[:],
                                         start=(half == 0), stop=(half == 1))
                    nc.scalar.activation(lse[:, sl], se_ps[:], AF.Ln)
                    pk_ps = psp.tile([1, 512], F32, tag="pps")
                    nc.tensor.matmul(pk_ps[:], ones65[:], paug[:, sl],
                                     start=True, stop=True)
                    nc.vector.tensor_tensor(loss[:, sl], lse[:, sl], pk_ps[:],
                                            mybir.AluOpType.subtract)
            nc.vector.tensor_scalar_mul(loss[:], loss[:],
                                        float(1.0 / np.log(2.0)))
            nc.default_dma_engine.dma_start(out_d[:], loss[:])
            if dbg:
                nc.default_dma_engine.dma_start(outm_d[:], m_seq[:])
                nc.default_dma_engine.dma_start(outp_d[:], preC[:])

    nc.compile()
    return nc


def _prep_core_inputs(x0, emb, W_enc_w, W_enc_b, W_dec_w, W_dec_b,
                      b_enc_w, b_enc_b, b_dec_w, b_dec_b, dec_w, dec_b):
    """Host-side gathers/packing -> (flags, list of per-core input dicts)."""
    f32 = np.float32
    x0 = np.asarray(x0)
    xp = np.concatenate([np.zeros((B, L), x0.dtype), x0], axis=1)  # [B, N+L]
    e = np.asarray(emb, f32)[xp]  # [B, N+L, E]

    has_wdb = bool(np.any(np.asarray(W_dec_b)))
    has_bdb = bool(np.any(np.asarray(b_dec_b)))
    has_decb = bool(np.any(np.asarray(dec_b)))

    # shared weight packs
    Wcat = np.concatenate([np.asarray(W_enc_w, f32), np.asarray(b_enc_w, f32)],
                          axis=1)  # [1088, 128]
    wpre1 = np.zeros((128, 8 * 128), f32)
    wpre2 = np.zeros((128, 8 * 64), f32)
    for c in range(8):
        blk = Wcat[64 + 128 * c: 64 + 128 * (c + 1)]  # [128, 128]
        wpre1[:, c * 128: c * 128 + 64] = blk[:, :64]
        wpre1[:, c * 128 + 64: c * 128 + 128] = blk[:, :64]
        wpre2[:, c * 64:(c + 1) * 64] = blk[:, 64:]
    bias1 = np.concatenate([np.asarray(W_enc_b, f32)] * 2).reshape(1, 128)
    bias2 = np.asarray(b_enc_b, f32).reshape(1, 64)
    a1b1 = np.zeros((128, 192), f32)
    a1b1[0:64, 0:128] = np.concatenate([Wcat[:64, :64]] * 2, axis=1)
    a1b1[0:64, 128:192] = Wcat[:64, 64:]

    W2r = np.asarray(W_dec_w, f32).reshape(H, M, M)  # [h, i, j]
    wstatT = np.zeros((128, 16 * 128), f32)
    for p2 in range(16):
        for half, c in ((0, 2 * p2), (1, 2 * p2 + 1)):
            rows = slice(64 * half, 64 * half + 64)
            wstatT[rows, p2 * 128: p2 * 128 + 64] = W2r[:, c, :].T
            wstatT[rows, p2 * 128 + 64: p2 * 128 + 128] = W2r[:, c + 32, :].T
    bwdup = np.concatenate([np.asarray(b_dec_w, f32)] * 2, axis=1)  # [64,128]

    shared = dict(
        wpre1=wpre1.astype(BF16), wpre2=wpre2.astype(BF16),
        bias1=bias1.astype(BF16), bias2=bias2.astype(BF16),
        a1b1=a1b1.astype(F8),
        wstatT=wstatT.astype(F8),
        ident=np.eye(128, dtype=f32).astype(BF16),
        bwdup=bwdup.astype(BF16),
        decbf=np.asarray(dec_w, f32).astype(BF16),
        ones65=np.ones((65, 1), f32).astype(BF16),
        ones128=np.ones((128, 1), f32).astype(BF16),
    )
    if has_wdb:
        shared["wbdup"] = np.concatenate(
            [np.asarray(W_dec_b, f32).reshape(M, M).T] * 2, axis=1).astype(BF16)
    if has_bdb:
        shared["bdb"] = np.asarray(b_dec_b, f32).reshape(64, 1)
    if has_decb:
        shared["decb"] = np.asarray(dec_b, f32).reshape(2, 128).T.copy()

    in_maps = []
    dec_wT = np.asarray(dec_w, f32).T.copy()  # [256, 64]
    dec_bv = np.asarray(dec_b, f32)
    for k in range(NCORES):
        rows = slice(BL * k, BL * (k + 1))
        ek = e[rows]  # [BL, N+L, E]
        # e8[l_sub*16+eps, tau*BL+b] = ek[b, tau+l_sub, eps]
        e8 = np.zeros((128, E8COLS), f32)
        for ls in range(8):
            blk = ek[:, ls: ls + TAU, :].transpose(2, 1, 0)  # [E, TAU, BL]
            e8[ls * 16:(ls + 1) * 16] = blk.reshape(E, E8COLS)
        y = np.asarray(x0[rows])  # [BL, N]
        g = dec_wT[y]  # [BL, N, 64]
        gaug = np.zeros((65, NB), f32)
        gaug[:64] = g.transpose(2, 1, 0).reshape(64, NB)
        gaug[64] = dec_bv[y].T.reshape(NB)
        d = dict(shared)
        d["e8"] = e8.astype(BF16)
        d["gaug"] = gaug.astype(BF16)
        in_maps.append(d)
    return (has_wdb, has_bdb, has_decb), in_maps


def kernel(**inputs):
    flags, in_maps = _prep_core_inputs(**inputs)
    key = ("nc",) + flags
    if key not in _cache:
        _cache[key] = _build_nc(has_wdb=flags[0], has_bdb=flags[1],
                                has_decb=flags[2])
    nc = _cache[key]
    res = run_bass_kernel_spmd(nc, in_maps, list(range(NCORES)),
                               trace=bool(os.environ.get("KERNEL_TRACE")))
    _cache["last_result"] = res
    out = np.zeros((N, B), np.float32)
    for k in range(NCORES):
        out[:, BL * k: BL * (k + 1)] = res.results[k]["out"].reshape(N, BL)
    return out.reshape(-1)


# revision 15
# speedup vs baseline: 1.0145x; 1.0145x over previous
"""Trainium2 Bass kernel for nn_Net4 (hypernetwork RNN scan).

Model (per step t, per batch row b):
  h1 = sigmoid(m @ A1 + pre1[t])          A1 = W_enc_w[:64]
  h2 = sigmoid(m @ B1 + pre2[t])          B1 = b_enc_w[:64]
  Wm = (h1 @ W_dec_w + W_dec_b).reshape(64,64)
  bm = h2 @ b_dec_w + b_dec_b
  m' = sigmoid(Wm @ m + bm)
  loss[t] = (logsumexp(m'@dec_w+dec_b) - (m'@dec_w+dec_b)[y]) / ln2

pre1/pre2 are the window-dependent parts, precomputed on device via a
shifted-embedding matmul, stored interleaved in preC (bf16).  The
bilinear Wm@m is reassociated as
  a[b,i] = sum_h h1[b,h] * T[b,h,i],  T[b,h,i] = sum_j W2r[h,i,j] m[b,j]

Scan critical cycle (all bf16):
  sigma(a) -> m_seq -> {g-MMs, 32 T-MMs} -> T copies -> a-MMs -> sigma(a)
The pre1/pre2 bias is injected into g_ps by an identity matmul (so one
sigmoid covers h1-dup + h2), and a_ps is duplicated into both partition
halves by col-tiled twin a-MMs (so one sigmoid produces the duplicated
m layout the next step's matmuls need).

Sharding: batch rows 2k,2k+1 -> core k; zero cross-core communication.
"""

import os
import sys
import numpy as np

sys.path.insert(0, "/opt/trn_rl_repo")

import concourse.bass as bass
import concourse.bacc as bacc
import concourse.mybir as mybir
import concourse.tile as tile
from concourse.bass_utils import run_bass_kernel_spmd

import ml_dtypes

BF16 = ml_dtypes.bfloat16
F8 = ml_dtypes.float8_e4m3

Cin, E, L, M, H, Cout = 256, 16, 64, 64, 64, 256
B, N = 16, 2048
D = M + L * E  # 1088
NCORES = 8
BL = B // NCORES  # 2 batch rows per core
NB = N * BL       # 4096 (t,b) pairs per core
TAU = N + L - 8   # e8 time length: tau in [0, 2104)
E8COLS = TAU * BL  # 4208

F32 = mybir.dt.float32
BF16_DT = mybir.dt.bfloat16
F8_DT = mybir.dt.float8e4
AF = mybir.ActivationFunctionType

_cache = {}


def _build_nc(unroll=16, staggered=False, has_wdb=False, has_bdb=False,
              has_decb=False, sim_closers=None):
    if sim_closers is None:
        sim_closers = bool(os.environ.get("KERNEL_SIM_CLOSERS"))
    nc = bacc.Bacc("TRN2", target_bir_lowering=False, debug=True)

    # ---- DRAM parameters (per-core inputs) ----
    def P(name, shape, dt):
        return nc.declare_dram_parameter(name, list(shape), dt, isOutput=False)

    e8_d = P("e8", (128, E8COLS), BF16_DT)
    wpre1_d = P("wpre1", (128, 8 * 128), BF16_DT)
    wpre2_d = P("wpre2", (128, 8 * 64), BF16_DT)
    bias1_d = P("bias1", (1, 128), BF16_DT)   # [W_enc_b | W_enc_b]
    bias2_d = P("bias2", (1, 64), BF16_DT)    # b_enc_b
    a1b1_d = P("a1b1", (128, 192), F8_DT)
    wstatT_d = P("wstatT", (128, 16 * 128), F8_DT)
    ident_d = P("ident", (128, 128), BF16_DT)
    bwdup_d = P("bwdup", (64, 128), BF16_DT)  # [b_dec_w | b_dec_w]
    dec_d = P("decbf", (64, 256), BF16_DT)    # dec_w
    gaug_d = P("gaug", (65, NB), BF16_DT)     # dec_wT[y] rows + dec_b[y] row
    ones65_d = P("ones65", (65, 1), BF16_DT)
    ones128_d = P("ones128", (128, 1), BF16_DT)
    if has_wdb:
        wbdup_d = P("wbdup", (64, 128), BF16_DT)  # W_dec_b resh [j,i] dup
    if has_bdb:
        bdb_d = P("bdb", (64, 1), F32)        # b_dec_b (sigmoid bias)
    if has_decb:
        decb_d = P("decb", (128, 2), F32)     # dec_b halves (exp bias)
    out_d = nc.declare_dram_parameter("out", [1, NB], F32, isOutput=True)
    dbg = bool(os.environ.get("KERNEL_DEBUG"))
    if dbg:
        outm_d = nc.declare_dram_parameter("outm", [64, NB + BL], BF16_DT, isOutput=True)
        outp_d = nc.declare_dram_parameter("outp", [128, N, 4], BF16_DT, isOutput=True)

    with tile.TileContext(nc) as tc:
        with (
            tc.tile_pool(name="persist", bufs=1) as pp,
            tc.tile_pool(name="psum", bufs=2, space="PSUM") as psp,
        ):
            e8 = pp.tile([128, E8COLS], BF16_DT)
            wpre1 = pp.tile([128, 8 * 128], BF16_DT)
            wpre2 = pp.tile([128, 8 * 64], BF16_DT)
            bias1 = pp.tile([1, 128], BF16_DT)
            bias2 = pp.tile([1, 64], BF16_DT)
            a1b1 = pp.tile([128, 192], F8_DT)
            wstatT = pp.tile([128, 16 * 128], F8_DT)
            ident = pp.tile([128, 128], BF16_DT)
            bwdup = pp.tile([64, 128], BF16_DT)
            decbf = pp.tile([64, 256], BF16_DT)
            gaug = pp.tile([65, NB], BF16_DT)
            ones65 = pp.tile([65, 1], BF16_DT)
            ones128 = pp.tile([128, 1], BF16_DT)

            loads = [
                (e8, e8_d), (wpre1, wpre1_d), (wpre2, wpre2_d),
                (bias1, bias1_d), (bias2, bias2_d), (a1b1, a1b1_d),
                (wstatT, wstatT_d), (ident, ident_d), (bwdup, bwdup_d),
                (decbf, dec_d), (gaug, gaug_d),
                (ones65, ones65_d), (ones128, ones128_d),
            ]
            if has_wdb:
                wbdup = pp.tile([64, 128], BF16_DT)
                loads.append((wbdup, wbdup_d))
            if has_bdb:
                bdb = pp.tile([64, 1], F32)
                loads.append((bdb, bdb_d))
            if has_decb:
                decb = pp.tile([128, 2], F32)
                loads.append((decb, decb_d))
            for sb, dr in loads:
                nc.default_dma_engine.dma_start(sb[:], dr[:])

            # preC: per step 4 cols: [pre1_dup(2) | pre2(2)] (bf16)
            preC = pp.tile([128, N, 4], BF16_DT)
            m_seq = pp.tile([64, NB + BL], BF16_DT)
            m_blk = pp.tile([128, 4], F8_DT)
            onerow = pp.tile([1, 512], BF16_DT)
            zrow = pp.tile([1, 128], BF16_DT)
            scrap = pp.tile([1, 2], F32)

            nc.vector.memset(m_seq[:, 0:BL], 0.0)
            nc.vector.memset(m_blk[:], 0.0)
            nc.vector.memset(preC[64:128, :, 2:4], 0.0)
            nc.vector.memset(onerow[:], 1.0)
            nc.vector.memset(zrow[:], 0.0)
            nc.vector.memset(scrap[:], 0.0)

            # ---- precompute pre1/pre2 -> preC (interleaved, bf16) ----
            for n in range(8):
                ps1 = psp.tile([128, 256, 2], F32, tag="pps")
                nc.tensor.matmul(ps1[:], bias1[:], onerow[:],
                                 start=True, stop=False)
                for c in range(8):
                    nc.tensor.matmul(
                        ps1[:],
                        wpre1[:, c * 128:(c + 1) * 128],
                        e8[:, 16 * c + 512 * n: 16 * c + 512 * n + 512],
                        start=False, stop=(c == 7),
                    )
                nc.vector.tensor_copy(
                    preC[:, 256 * n: 256 * (n + 1), 0:2], ps1[:])
                ps2 = psp.tile([64, 256, 2], F32, tag="pps")
                nc.tensor.matmul(ps2[:], bias2[:], onerow[:],
                                 start=True, stop=False)
                for c in range(8):
                    nc.tensor.matmul(
                        ps2[:],
                        wpre2[:, c * 64:(c + 1) * 64],
                        e8[:, 16 * c + 512 * n: 16 * c + 512 * n + 512],
                        start=False, stop=(c == 7),
                    )
                nc.vector.tensor_copy(
                    preC[0:64, 256 * n: 256 * (n + 1), 2:4], ps2[:])

            # dummy sigmoid so the act-table load is hoisted out of the loop
            nc.scalar.activation(scrap[:], scrap[:], AF.Sigmoid)

            # ---- the scan ----
            with (
                tc.tile_pool(name="scan_sb", bufs=2) as wp,
                tc.For_i(0, N, unroll, staggered_reset=staggered,
                         hint_engines=(mybir.EngineType.PE,)) as iv,
            ):
                for k in range(unroll):
                    tcol = (iv + k) * BL
                    g_ps = psp.tile([128, 4], F32, tag="g_ps", bufs=1)
                    T_psA = psp.tile([128, 9, 4], F32, tag="T_psA", bufs=1)
                    T_psB = psp.tile([128, 6, 4], F32, tag="T_psB", bufs=1)
                    T_psC = psp.tile([128, 1, 4], F32, tag="T_psC", bufs=1)
                    a_ps = psp.tile([64, BL], F32, tag="a_ps")
                    hbuf = wp.tile([128, 4], BF16_DT, tag="hbuf")
                    tsb = wp.tile([128, 16, 4], BF16_DT, tag="tsb")

                    # g = pre (identity MM) + [A1|A1]m (rows 0-63 dup) + A2 m
                    nc.tensor.matmul(g_ps[:], ident[:],
                                     preC[:, bass.ds(iv + k, 1), :],
                                     start=True, stop=False)
                    nc.tensor.matmul(g_ps[0:64, 2:4], a1b1[0:64, 128:192],
                                     m_blk[0:64, 0:2],
                                     start=False, stop=False,
                                     skip_group_check=True,
                                     tile_position=(0, 0))
                    nc.tensor.matmul(g_ps[:, 0:2], a1b1[0:64, 0:128],
                                     m_blk[0:64, 0:2],
                                     start=False, stop=not sim_closers,
                                     skip_group_check=True,
                                     tile_position=(0, 0))
                    if sim_closers:
                        nc.tensor.matmul(g_ps[:], zrow[:, 0:128],
                                         onerow[:, 0:4],
                                         start=False, stop=True)
                    # T chunks: one full-array MM per 128x128 stationary;
                    # rhs is block-diag so out cols 0:2 = even chunk (rows
                    # 0-63 of wstatT), cols 2:4 = odd chunk (rows 64-127)
                    for p2 in range(16):
                        Tp, pi = ((T_psA, p2) if p2 < 9 else
                                  (T_psB, p2 - 9) if p2 < 15 else
                                  (T_psC, p2 - 15))
                        nc.tensor.matmul(
                            Tp[:, pi, :],
                            wstatT[:, p2 * 128:(p2 + 1) * 128],
                            m_blk[:],
                            start=True, stop=True)
                    # h = sigmoid(g) : [h1 dup | h2] in one shot
                    nc.scalar.activation(hbuf[:], g_ps[:], AF.Sigmoid)
                    # T -> SBUF (bf16), layout-preserving
                    nc.vector.tensor_copy(tsb[:, 0:9, :], T_psA[:])
                    nc.vector.tensor_copy(tsb[:, 9:15, :], T_psB[:])
                    nc.vector.tensor_copy(tsb[:, 15:16, :], T_psC[:])
                    # a = bw@h2 (+ wb@m) + sum_h h1*T, dup'd into both halves
                    nc.tensor.matmul(a_ps[:], bwdup[:, 0:64], hbuf[0:64, 2:4],
                                     start=True, stop=False)
                    if has_wdb:
                        nc.tensor.matmul(a_ps[:], wbdup[:, 0:64],
                                         m_blk[0:64, 0:2],
                                         start=False, stop=False)
                    for b in range(BL):
                        nc.tensor.matmul(a_ps[0:32, b: b + 1],
                                         tsb[0:64, :, b:4:2],
                                         hbuf[0:64, b: b + 1],
                                         start=False, stop=False,
                                         skip_group_check=True,
                                         tile_position=(0, 0))
                        nc.tensor.matmul(a_ps[32:64, b: b + 1],
                                         tsb[64:128, :, b:4:2],
                                         hbuf[64:128, b: b + 1],
                                         start=False,
                                         stop=(b == BL - 1 and not sim_closers),
                                         skip_group_check=True,
                                         tile_position=(64, 32))
                    if sim_closers:
                        nc.tensor.matmul(a_ps[:], zrow[:, 0:64], onerow[:, 0:2],
                                         start=False, stop=True)
                    # m' = sigmoid(a), duplicated halves in one call
                    if has_bdb:
                        nc.scalar.activation(m_blk[0:64, 0:2], a_ps[:],
                                             AF.Sigmoid, bias=bdb[0:64, :])
                    else:
                        nc.scalar.activation(m_blk[0:64, 0:2], a_ps[:],
                                             AF.Sigmoid)
                    nc.vector.tensor_copy(m_blk[64:128, 2:4], m_blk[0:64, 0:2])
                    nc.vector.tensor_copy(m_seq[:, bass.ds(tcol + BL, BL)],
                                          m_blk[0:64, 0:2])

            # ---- bulk loss ----
            lse = pp.tile([1, NB], F32)
            paug = pp.tile([65, NB], BF16_DT)
            loss = pp.tile([1, NB], F32)
            nc.vector.tensor_copy(paug[64:65, :], gaug[64:65, :])
            nc.vector.tensor_tensor(paug[0:64, :], gaug[0:64, :],
                                    m_seq[:, BL: NB + BL],
                                    mybir.AluOpType.mult)
            with tc.tile_pool(name="bulk", bufs=2) as bp:
                for tcn in range(8):
                    sl = slice(512 * tcn, 512 * (tcn + 1))
                    se_ps = psp.tile([1, 512], F32, tag="pps")
                    for half in range(2):
                        lg_ps = psp.tile([128, 512], F32, tag="pps")
                        exps = bp.tile([128, 512], BF16_DT, tag="exps")
                        nc.tensor.matmul(
                            lg_ps[:],
                            decbf[:, half * 128:(half + 1) * 128],
                            m_seq[:, BL + 512 * tcn: BL + 512 * (tcn + 1)],
                            start=True, stop=True)
                        if has_decb:
                            nc.scalar.activation(exps[:], lg_ps[:], AF.Exp,
                                                 bias=decb[:, half:half + 1])
                        else:
                            nc.scalar.activation(exps[:], lg_ps[:], AF.Exp)
                        nc.tensor.matmul(se_ps[:], ones128[:], exps[:],
                                         start=(half == 0), stop=(half == 1))
                    nc.scalar.activation(lse[:, sl], se_ps[:], AF.Ln)
                    pk_ps = psp.tile([1, 512], F32, tag="pps")
                    nc.tensor.matmul(pk_ps[:], ones65[:], paug[:, sl],
                                     start=True, stop=True)
                    nc.vector.tensor_tensor(loss[:, sl], lse[:, sl], pk_ps[:],
                                            mybir.AluOpType.subtract)
            nc.vector.tensor_scalar_mul(loss[:], loss[:],
                                        float(1.0 / np.log(2.0)))
            nc.default_dma_engine.dma_start(out_d[:], loss[:])
            if dbg:
                nc.default_dma_engine.dma_start(outm_d[:], m_seq[:])
                nc.default_dma_engine.dma_start(outp_d[:], preC[:])

    nc.compile()
    return nc


def _prep_core_inputs(x0, emb, W_enc_w, W_enc_b, W_dec_w, W_dec_b,
                      b_enc_w, b_enc_b, b_dec_w, b_dec_b, dec_w, dec_b):
    """Host-side gathers/packing -> (flags, list of per-core input dicts)."""
    f32 = np.float32
    x0 = np.asarray(x0)
    xp = np.concatenate([np.zeros((B, L), x0.dtype), x0], axis=1)  # [B, N+L]
    e = np.asarray(emb, f32)[xp]  # [B, N+L, E]

    has_wdb = bool(np.any(np.asarray(W_dec_b)))
    has_bdb = bool(np.any(np.asarray(b_dec_b)))
    has_decb = bool(np.any(np.asarray(dec_b)))

    # shared weight packs
    Wcat = np.concatenate([np.asarray(W_enc_w, f32), np.asarray(b_enc_w, f32)],
                          axis=1)  # [1088, 128]
    wpre1 = np.zeros((128, 8 * 128), f32)
    wpre2 = np.zeros((128, 8 * 64), f32)
    for c in range(8):
        blk = Wcat[64 + 128 * c: 64 + 128 * (c + 1)]  # [128, 128]
        wpre1[:, c * 128: c * 128 + 64] = blk[:, :64]
        wpre1[:, c * 128 + 64: c * 128 + 128] = blk[:, :64]
        wpre2[:, c * 64:(c + 1) * 64] = blk[:, 64:]
    bias1 = np.concatenate([np.asarray(W_enc_b, f32)] * 2).reshape(1, 128)
    bias2 = np.asarray(b_enc_b, f32).reshape(1, 64)
    a1b1 = np.zeros((128, 192), f32)
    a1b1[0:64, 0:128] = np.concatenate([Wcat[:64, :64]] * 2, axis=1)
    a1b1[0:64, 128:192] = Wcat[:64, 64:]

    W2r = np.asarray(W_dec_w, f32).reshape(H, M, M)  # [h, i, j]
    wstatT = np.zeros((128, 16 * 128), f32)
    for p2 in range(16):
        for half, c in ((0, 2 * p2), (1, 2 * p2 + 1)):
            rows = slice(64 * half, 64 * half + 64)
            wstatT[rows, p2 * 128: p2 * 128 + 64] = W2r[:, c, :].T
            wstatT[rows, p2 * 128 + 64: p2 * 128 + 128] = W2r[:, c + 32, :].T
    bwdup = np.concatenate([np.asarray(b_dec_w, f32)] * 2, axis=1)  # [64,128]

    shared = dict(
        wpre1=wpre1.astype(BF16), wpre2=wpre2.astype(BF16),
        bias1=bias1.astype(BF16), bias2=bias2.astype(BF16),
        a1b1=a1b1.astype(F8),
        wstatT=wstatT.astype(F8),
        ident=np.eye(128, dtype=f32).astype(BF16),
        bwdup=bwdup.astype(BF16),
        decbf=np.asarray(dec_w, f32).astype(BF16),
        ones65=np.ones((65, 1), f32).astype(BF16),
        ones128=np.ones((128, 1), f32).astype(BF16),
    )
    if has_wdb:
        shared["wbdup"] = np.concatenate(
            [np.asarray(W_dec_b, f32).reshape(M, M).T] * 2, axis=1).astype(BF16)
    if has_bdb:
        shared["bdb"] = np.asarray(b_dec_b, f32).reshape(64, 1)
    if has_decb:
        shared["decb"] = np.asarray(dec_b, f32).reshape(2, 128).T.copy()

    in_maps = []
    dec_wT = np.asarray(dec_w, f32).T.copy()  # [256, 64]
    dec_bv = np.asarray(dec_b, f32)
    for k in range(NCORES):
        rows = slice(BL * k, BL * (k + 1))
        ek = e[rows]  # [BL, N+L, E]
        # e8[l_sub*16+eps, tau*BL+b] = ek[b, tau+l_sub, eps]
        e8 = np.zeros((128, E8COLS), f32)
        for ls in range(8):
            blk = ek[:, ls: ls + TAU, :].transpose(2, 1, 0)  # [E, TAU, BL]
            e8[ls * 16:(ls + 1) * 16] = blk.reshape(E, E8COLS)
        y = np.asarray(x0[rows])  # [BL, N]
        g = dec_wT[y]  # [BL, N, 64]
        gaug = np.zeros((65, NB), f32)
        gaug[:64] = g.transpose(2, 1, 0).reshape(64, NB)
        gaug[64] = dec_bv[y].T.reshape(NB)
        d = dict(shared)
        d["e8"] = e8.astype(BF16)
        d["gaug"] = gaug.astype(BF16)
        in_maps.append(d)
    return (has_wdb, has_bdb, has_decb), in_maps


def kernel(**inputs):
    flags, in_maps = _prep_core_inputs(**inputs)
    key = ("nc",) + flags
    if key not in _cache:
        _cache[key] = _build_nc(has_wdb=flags[0], has_bdb=flags[1],
                                has_decb=flags[2])
    nc = _cache[key]
    res = run_bass_kernel_spmd(nc, in_maps, list(range(NCORES)),
                               trace=bool(os.environ.get("KERNEL_TRACE")))
    _cache["last_result"] = res
    out = np.zeros((N, B), np.float32)
    for k in range(NCORES):
        out[:, BL * k: BL * (k + 1)] = res.results[k]["out"].reshape(N, BL)
    return out.reshape(-1)
